# revision 26
# baseline (speedup 1.0000x reference)
"""Additive-attention pooling kernel for one TRN2 chip (8 NeuronCores).

reference:
    score = tanh(x @ W1 + b) @ W2        # [B, S, 1]
    alpha = softmax(score, axis=1)       # softmax over S
    ctx   = sum_s x * alpha              # [B, D]
    returns (ctx, alpha)

Shapes: x [256, 2048, 128] f32, W1 [128, 64], W2 [64, 1], b [64].
Sharding: data-parallel over batch — 32 samples per core, no collectives.

Per-core plan (B_local=32 processed as 8 QUADS of samples, b = 4*qi + jq).
Block layout: s = 16*p + r (partition p holds 16 consecutive rows), giving
8KB-contiguous DMA runs.
  - SWDGE cast-DMA loads x f32->bf16: xn[p, jq, r, d]
  - bf16 written back to DRAM scratch in [(jq s), d] row order, then one
    big xbar transposed load per quad: xt[d, jq, s].  All transposes stay
    on nc.sync: concurrent xbar transposes on different HWDGE queues
    corrupt each other (shared xbar state).
  - h = W1.T @ xT on PE, two samples packed per 128-partition psum tile
  - tanh(+b) on ACT -> bf16 th[u2, pair, s]
  - score: lhsT = th block-row slice (stride-16 cols), rhs = W2-stacked
    -> psc[:, 4r+2pi : 4r+2pi+2]; col (r, jq) <-> score[4qi+jq, 16p+r]
  - exp on ACT (no max subtraction: |score| <= sum|W2| ~ 7, safe in f32)
  - denominators via ones-matmuls + DVE reciprocal, broadcast matmul
  - alpha = e * recip (DVE tensor_scalar, f32 out + bf16 pooling copy)
  - pooling: lhsT = alpha cols [128, 4] bf16, rhs = xn[:, :, r, :]
    [128, 512] -> psum [4, 512], diagonal 128-blocks valid
  - outputs staged in SBUF, DMAed once at the end
"""

import sys
import types

import numpy as np
import ml_dtypes

import concourse.bass as bass
import concourse.tile as tile
from concourse.tile import add_dep_helper
from concourse import bacc, mybir
from concourse.bass_utils import run_bass_kernel_spmd


def _ensure_ntff_hook():
    """The image's antenv lacks axon_hooks, so trn_boot's NTFF profile hook
    registration degraded silently. Recreate the module and register the
    ctypes-based hook so run_bass_kernel_spmd(trace=True) can profile."""
    if "antenv.axon_hooks" in sys.modules:
        return
    mod = types.ModuleType("antenv.axon_hooks")
    mod._hook = None

    def set_axon_ntff_profile_hook(h):
        mod._hook = h

    def get_axon_ntff_profile_hook():
        return mod._hook

    mod.set_axon_ntff_profile_hook = set_axon_ntff_profile_hook
    mod.get_axon_ntff_profile_hook = get_axon_ntff_profile_hook
    sys.modules["antenv.axon_hooks"] = mod
    try:
        import antenv
        antenv.axon_hooks = mod
        from trn_agent_boot.trn_boot import _ntff_profile_via_ctypes
        mod._hook = _ntff_profile_via_ctypes("/opt/axon/libaxon_pjrt.so")
    except Exception:
        pass


_ensure_ntff_hook()

F32 = mybir.dt.float32
BF16 = mybir.dt.bfloat16

N_CORES = 8
B, S, D, U = 256, 2048, 128, 64
B_LOCAL = B // N_CORES          # 32
N_PAIRS = B_LOCAL // 2          # 16
NQ = 4                          # 4 chunks of 512 per sample
R = 16                          # rows per partition block; s = 16*p + r


def build_graph(n_pairs=N_PAIRS):
    assert n_pairs % 2 == 0, "quad layout needs even n_pairs"
    n_quads = n_pairs // 2
    nc = bacc.Bacc("TRN2", target_bir_lowering=False, debug=False,
                   num_devices=N_CORES)
    b_local = 4 * n_quads

    x_d = nc.dram_tensor("x", [b_local, S, D], F32, kind="ExternalInput").ap()
    w1_d = nc.dram_tensor("w1", [D, U], BF16, kind="ExternalInput").ap()
    w2s_d = nc.dram_tensor("w2s", [2 * U, 2], BF16, kind="ExternalInput").ap()
    bb_d = nc.dram_tensor("bb", [2 * U, 1], F32, kind="ExternalInput").ap()
    onc_d = nc.dram_tensor("onc", [128, 1], F32, kind="ExternalInput").ap()
    onr_d = nc.dram_tensor("onr", [1, 128], F32, kind="ExternalInput").ap()

    ctx_d = nc.dram_tensor("ctx", [b_local, D], F32, kind="ExternalOutput").ap()
    alp_d = nc.dram_tensor("alpha", [b_local, S], F32, kind="ExternalOutput").ap()

    with tile.TileContext(nc) as tc:
        with (
            tc.tile_pool(name="consts", bufs=1) as consts,
            tc.tile_pool(name="xn_p", bufs=5) as xn_p,
            tc.tile_pool(name="xt_p", bufs=2) as xt_p,
            tc.tile_pool(name="scr_p", bufs=3, space="DRAM") as scr_p,
            tc.tile_pool(name="th_p", bufs=2) as th_p,
            tc.tile_pool(name="sm_p", bufs=3) as sm_p,
            tc.tile_pool(name="out_p", bufs=1) as out_p,
            tc.tile_pool(name="ph_p", bufs=3, space=bass.MemorySpace.PSUM) as ph_p,
            tc.tile_pool(name="psc_p", bufs=2, space=bass.MemorySpace.PSUM) as psc_p,
            tc.tile_pool(name="pmisc_p", bufs=1, space=bass.MemorySpace.PSUM) as pmisc_p,
        ):
            w1_sb = consts.tile([D, U], BF16)
            nc.scalar.dma_start(w1_sb[:], w1_d)
            w2s_sb = consts.tile([2 * U, 2], BF16)
            nc.scalar.dma_start(w2s_sb[:], w2s_d)
            bb_sb = consts.tile([2 * U, 1], F32)
            nc.scalar.dma_start(bb_sb[:], bb_d)
            onc_sb = consts.tile([128, 1], F32)
            nc.scalar.dma_start(onc_sb[:], onc_d)
            onr_sb = consts.tile([1, 128], F32)
            nc.scalar.dma_start(onr_sb[:], onr_d)

            # ctx staged as [4, n_quads*512]: per quad a [4, 512] block whose
            # valid parts are row jq, cols jq*128:(jq+1)*128
            ctx_all = out_p.tile([4, n_quads * 4 * D], F32)
            # normalized alpha, [p, jq, (qi r)] -> alpha[b=4qi+jq, s=16p+r]
            alpha_all = out_p.tile([128, 4, n_quads * R], F32)

            # DRAM scratch deps are not tracked by Tile; enforce manually.
            scr_last_tp = [None] * 3

            for i in range(n_quads):
                b0 = 4 * i

                # ---- load x quad, cast f32->bf16, block layout ----
                # xn[p, jq, r, d] = x[b0+jq, 16p+r, d]  (8KB contiguous runs)
                xn = xn_p.tile([128, 4, R, D], BF16, name="xn", tag="xn")
                src = x_d[b0:b0 + 4].rearrange("j (p r) d -> p j r d", r=R)
                nc.gpsimd.dma_start(xn[:], src)

                # ---- transposed tiles for the score path ----
                scr = scr_p.tile([4 * S, D], BF16, name="scr", tag="scr")
                wb_inst = nc.scalar.dma_start(
                    scr.rearrange("(j p r) d -> p j r d", j=4, r=R), xn[:])
                if scr_last_tp[i % 3] is not None:
                    add_dep_helper(wb_inst.ins, scr_last_tp[i % 3],
                                   reason="scratch WAR: wb after prev tp")
                xt = xt_p.tile([128, 4, S], BF16, name="xt", tag="xt")
                tp_inst = nc.sync.dma_start_transpose(
                    xt.rearrange("d j s -> d (j s)"), scr[:])
                add_dep_helper(tp_inst.ins, wb_inst.ins,
                               reason="scratch RAW: tp after wb")
                scr_last_tp[i % 3] = tp_inst.ins

                # ---- h = x@W1 (as hT pairs), tanh ----
                # th[u2, pi, s] = tanh(h) for samples jq = 2*pi, 2*pi+1
                th = th_p.tile([128, 2, S], BF16, name="th", tag="th")
                for pi in range(2):
                    for q in range(NQ):
                        ph = ph_p.tile([128, 512], F32, name="ph", tag="ph")
                        nc.tensor.matmul(ph[0:U, :], w1_sb[:],
                                         xt[:, 2 * pi, 512 * q:512 * (q + 1)],
                                         start=True, stop=True)
                        nc.tensor.matmul(ph[U:2 * U, :], w1_sb[:],
                                         xt[:, 2 * pi + 1, 512 * q:512 * (q + 1)],
                                         start=True, stop=True)
                        nc.scalar.activation(
                            th[:, pi, 512 * q:512 * (q + 1)],
                            ph[:], mybir.ActivationFunctionType.Tanh,
                            bias=bb_sb[:])

                # ---- score matmuls: one per (pair, block-row r) ----
                # psc col (r, jq) = 4r + jq
                psc = psc_p.tile([128, 4 * R], F32, name="psc", tag="psc")
                for pi in range(2):
                    thr = th.rearrange("u pi (z r) -> u pi r z", r=R)
                    for r in range(R):
                        nc.tensor.matmul(psc[:, 4 * r + 2 * pi:4 * r + 2 * pi + 2],
                                         thr[:, pi, r, :], w2s_sb[:],
                                         start=True, stop=True)

                # ---- exp, per-sample denominators ----
                e_sb = sm_p.tile([128, 4 * R], F32, name="e_sb", tag="e_sb")
                nc.scalar.activation(e_sb[:], psc[:],
                                     mybir.ActivationFunctionType.Exp)
                cs = sm_p.tile([128, 4], F32, name="cs", tag="cs")
                e_v = e_sb.rearrange("p (r j) -> p r j", j=4)
                for j in range(4):
                    nc.vector.reduce_sum(cs[:, j:j + 1], e_v[:, :, j],
                                         axis=mybir.AxisListType.X)
                pden = pmisc_p.tile([1, 4], F32, name="pden", tag="pden")
                nc.tensor.matmul(pden[:], onc_sb[:], cs[:], start=True, stop=True)
                rec = sm_p.tile([1, 4], F32, name="rec", tag="rec")
                nc.vector.reciprocal(rec[:], pden[:])
                prb = pmisc_p.tile([128, 4], F32, name="prb", tag="prb")
                nc.tensor.matmul(prb[:], onr_sb[:], rec[:], start=True, stop=True)
                rb = sm_p.tile([128, 4], F32, name="rb", tag="rb")
                nc.vector.tensor_copy(rb[:], prb[:])

                # ---- normalize alpha (f32 output + bf16 pooling weights) ----
                alpha_b = sm_p.tile([128, 4 * R], BF16, name="alpha_b",
                                    tag="alpha_b")
                a_v = alpha_b.rearrange("p (r j) -> p r j", j=4)
                for j in range(4):
                    nc.vector.tensor_scalar_mul(
                        alpha_all[:, j, i * R:(i + 1) * R], e_v[:, :, j],
                        rb[:, j:j + 1])
                    nc.vector.tensor_scalar_mul(
                        a_v[:, :, j], e_v[:, :, j], rb[:, j:j + 1])

                # ---- pooling: ctx rows via alpha-as-weights matmuls ----
                pctx = pmisc_p.tile([4, 4 * D], F32, name="pctx", tag="pctx")
                for r in range(R):
                    nc.tensor.matmul(pctx[:], alpha_b[:, 4 * r:4 * r + 4],
                                     xn[:, :, r, :],
                                     start=(r == 0), stop=(r == R - 1))
                nc.vector.tensor_copy(
                    ctx_all[:, i * 4 * D:(i + 1) * 4 * D], pctx[:])

            # ---- alpha out: one DMA per jq over all quads ----
            for j in range(4):
                dst = alp_d.rearrange("(i j) (p r) -> j p i r",
                                      j=4, r=R)[j]
                nc.scalar.dma_start(dst, alpha_all[:, j, :].rearrange(
                    "p (i r) -> p i r", r=R))

            # ---- ctx out: per jq, pick the valid block of each quad ----
            for j in range(4):
                src = ctx_all.rearrange("r (i four d) -> r i four d",
                                        four=4, d=D)[j:j + 1, :, j, :]
                dst = ctx_d.rearrange("(i j) d -> j i d", j=4)[j:j + 1]
                nc.scalar.dma_start(dst, src)

    nc.compile()
    return nc


def make_in_maps(x, W1, W2, b, n_pairs=N_PAIRS, n_cores=N_CORES):
    x = np.asarray(x, dtype=np.float32)
    W1 = np.asarray(W1, dtype=np.float32)
    W2 = np.asarray(W2, dtype=np.float32)
    b = np.asarray(b, dtype=np.float32)

    w1 = W1.astype(ml_dtypes.bfloat16)
    w2s = np.zeros((2 * U, 2), dtype=ml_dtypes.bfloat16)
    w2s[0:U, 0] = W2[:, 0].astype(ml_dtypes.bfloat16)
    w2s[U:2 * U, 1] = W2[:, 0].astype(ml_dtypes.bfloat16)
    bb = np.concatenate([b, b]).reshape(2 * U, 1).astype(np.float32)
    onc = np.ones((128, 1), dtype=np.float32)
    onr = np.ones((1, 128), dtype=np.float32)

    b_local = 2 * n_pairs
    in_maps = []
    for ci in range(n_cores):
        in_maps.append({
            "x": np.ascontiguousarray(x[ci * b_local:(ci + 1) * b_local]),
            "w1": w1, "w2s": w2s, "bb": bb,
            "onc": onc, "onr": onr,
        })
    return in_maps


_CACHE = {}


def _get_graph(n_pairs=N_PAIRS):
    if n_pairs not in _CACHE:
        _CACHE[n_pairs] = build_graph(n_pairs)
    return _CACHE[n_pairs]


def run(x, W1, W2, b, trace=False, n_pairs=N_PAIRS, n_cores=N_CORES, **kw):
    nc = _get_graph(n_pairs)
    in_maps = make_in_maps(x, W1, W2, b, n_pairs, n_cores)
    res = run_bass_kernel_spmd(nc, in_maps, core_ids=list(range(n_cores)),
                               trace=trace, **kw)
    b_local = 2 * n_pairs
    ctx = np.concatenate([np.asarray(r["ctx"]) for r in res.results], axis=0)
    alpha = np.concatenate([np.asarray(r["alpha"]) for r in res.results], axis=0)
    return ctx, alpha.reshape(n_cores * b_local, S, 1), res


def kernel(x, W1, W2, b):
    ctx, alpha, _ = run(x, W1, W2, b)
    return ctx.astype(np.float32), alpha.astype(np.float32)


# revision 27
# speedup vs baseline: 1.1519x; 1.1519x over previous
"""Additive-attention pooling kernel for one TRN2 chip (8 NeuronCores).

reference:
    score = tanh(x @ W1 + b) @ W2        # [B, S, 1]
    alpha = softmax(score, axis=1)       # softmax over S
    ctx   = sum_s x * alpha              # [B, D]
    returns (ctx, alpha)

Shapes: x [256, 2048, 128] f32, W1 [128, 64], W2 [64, 1], b [64].
Sharding: data-parallel over batch — 32 samples per core, no collectives.

Per-core plan (B_local=32 processed as 8 QUADS of samples, b = 4*qi + jq).
Block layout: s = 16*p + r (partition p holds 16 consecutive rows), giving
8KB-contiguous DMA runs.
  - SWDGE cast-DMA loads x f32->bf16: xn[p, jq, r, d]
  - bf16 written back to DRAM scratch in [(jq s), d] row order, then one
    big xbar transposed load per quad: xt[d, jq, s].  All transposes stay
    on nc.sync: concurrent xbar transposes on different HWDGE queues
    corrupt each other (shared xbar state).
  - h = W1.T @ xT on PE, two samples packed per 128-partition psum tile
  - tanh(+b) on ACT -> bf16 th[u2, pair, s]
  - score: lhsT = th block-row slice (stride-16 cols), rhs = W2-stacked
    -> psc[:, 4r+2pi : 4r+2pi+2]; col (r, jq) <-> score[4qi+jq, 16p+r]
  - exp on ACT (no max subtraction: |score| <= sum|W2| ~ 7, safe in f32)
  - denominators via ones-matmuls + DVE reciprocal, broadcast matmul
  - alpha = e * recip (DVE tensor_scalar, f32 out + bf16 pooling copy)
  - pooling: lhsT = alpha cols [128, 4] bf16, rhs = xn[:, :, r, :]
    [128, 512] -> psum [4, 512], diagonal 128-blocks valid
  - outputs staged in SBUF, DMAed once at the end
"""

import sys
import types

import numpy as np
import ml_dtypes

import concourse.bass as bass
import concourse.tile as tile
from concourse.tile import add_dep_helper
from concourse import bacc, mybir
from concourse.bass_utils import run_bass_kernel_spmd


def _ensure_ntff_hook():
    """The image's antenv lacks axon_hooks, so trn_boot's NTFF profile hook
    registration degraded silently. Recreate the module and register the
    ctypes-based hook so run_bass_kernel_spmd(trace=True) can profile."""
    if "antenv.axon_hooks" in sys.modules:
        return
    mod = types.ModuleType("antenv.axon_hooks")
    mod._hook = None

    def set_axon_ntff_profile_hook(h):
        mod._hook = h

    def get_axon_ntff_profile_hook():
        return mod._hook

    mod.set_axon_ntff_profile_hook = set_axon_ntff_profile_hook
    mod.get_axon_ntff_profile_hook = get_axon_ntff_profile_hook
    sys.modules["antenv.axon_hooks"] = mod
    try:
        import antenv
        antenv.axon_hooks = mod
        from trn_agent_boot.trn_boot import _ntff_profile_via_ctypes
        mod._hook = _ntff_profile_via_ctypes("/opt/axon/libaxon_pjrt.so")
    except Exception:
        pass


_ensure_ntff_hook()

F32 = mybir.dt.float32
BF16 = mybir.dt.bfloat16

N_CORES = 8
B, S, D, U = 256, 2048, 128, 64
B_LOCAL = B // N_CORES          # 32
N_PAIRS = B_LOCAL // 2          # 16
NQ = 4                          # 4 chunks of 512 per sample
R = 16                          # rows per partition block; s = 16*p + r


def build_graph(n_pairs=N_PAIRS):
    assert n_pairs % 2 == 0, "quad layout needs even n_pairs"
    n_quads = n_pairs // 2
    nc = bacc.Bacc("TRN2", target_bir_lowering=False, debug=False,
                   num_devices=N_CORES)
    b_local = 4 * n_quads

    x_d = nc.dram_tensor("x", [b_local, S, D], F32, kind="ExternalInput").ap()
    w1_d = nc.dram_tensor("w1", [D, U], BF16, kind="ExternalInput").ap()
    w2s_d = nc.dram_tensor("w2s", [2 * U, 2], BF16, kind="ExternalInput").ap()
    bb_d = nc.dram_tensor("bb", [2 * U, 1], F32, kind="ExternalInput").ap()
    onc_d = nc.dram_tensor("onc", [128, 1], F32, kind="ExternalInput").ap()
    onr_d = nc.dram_tensor("onr", [1, 128], F32, kind="ExternalInput").ap()

    ctx_d = nc.dram_tensor("ctx", [b_local, D], F32, kind="ExternalOutput").ap()
    alp_d = nc.dram_tensor("alpha", [b_local, S], F32, kind="ExternalOutput").ap()

    with tile.TileContext(nc) as tc:
        with (
            tc.tile_pool(name="consts", bufs=1) as consts,
            tc.tile_pool(name="xn_p", bufs=4) as xn_p,
            tc.tile_pool(name="xt_p", bufs=3) as xt_p,
            tc.tile_pool(name="scr_p", bufs=3, space="DRAM") as scr_p,
            tc.tile_pool(name="th_p", bufs=3) as th_p,
            tc.tile_pool(name="sm_p", bufs=3) as sm_p,
            tc.tile_pool(name="out_p", bufs=1) as out_p,
            tc.tile_pool(name="ph_p", bufs=3, space=bass.MemorySpace.PSUM) as ph_p,
            tc.tile_pool(name="psc_p", bufs=2, space=bass.MemorySpace.PSUM) as psc_p,
            tc.tile_pool(name="pmisc_p", bufs=1, space=bass.MemorySpace.PSUM) as pmisc_p,
        ):
            w1_sb = consts.tile([D, U], BF16)
            nc.scalar.dma_start(w1_sb[:], w1_d)
            w2s_sb = consts.tile([2 * U, 2], BF16)
            nc.scalar.dma_start(w2s_sb[:], w2s_d)
            bb_sb = consts.tile([2 * U, 1], F32)
            nc.scalar.dma_start(bb_sb[:], bb_d)
            onc_sb = consts.tile([128, 1], F32)
            nc.scalar.dma_start(onc_sb[:], onc_d)
            onr_sb = consts.tile([1, 128], F32)
            nc.scalar.dma_start(onr_sb[:], onr_d)

            # ctx staged as [4, n_quads*512]: per quad a [4, 512] block whose
            # valid parts are row jq, cols jq*128:(jq+1)*128
            ctx_all = out_p.tile([4, n_quads * 4 * D], F32)
            # normalized alpha, [p, jq, (qi r)] -> alpha[b=4qi+jq, s=16p+r]
            alpha_all = out_p.tile([128, 4, n_quads * R], F32)

            # DRAM scratch deps are not tracked by Tile; enforce manually.
            scr_last_tp = [None] * 3

            for i in range(n_quads):
                b0 = 4 * i

                # ---- load x quad, cast f32->bf16, block layout ----
                # xn[p, jq, r, d] = x[b0+jq, 16p+r, d]  (8KB contiguous runs)
                xn = xn_p.tile([128, 4, R, D], BF16, name="xn", tag="xn")
                src = x_d[b0:b0 + 4].rearrange("j (p r) d -> p j r d", r=R)
                nc.gpsimd.dma_start(xn[:], src)

                # ---- transposed tiles for the score path ----
                scr = scr_p.tile([4 * S, D], BF16, name="scr", tag="scr")
                wb_inst = nc.scalar.dma_start(
                    scr.rearrange("(j p r) d -> p j r d", j=4, r=R), xn[:])
                if scr_last_tp[i % 3] is not None:
                    add_dep_helper(wb_inst.ins, scr_last_tp[i % 3],
                                   reason="scratch WAR: wb after prev tp")
                xt = xt_p.tile([128, 4, S], BF16, name="xt", tag="xt")
                tp_inst = nc.sync.dma_start_transpose(
                    xt.rearrange("d j s -> d (j s)"), scr[:])
                add_dep_helper(tp_inst.ins, wb_inst.ins,
                               reason="scratch RAW: tp after wb")
                scr_last_tp[i % 3] = tp_inst.ins

                # ---- h = x@W1 (as hT pairs), tanh ----
                # th[u2, pi, s] = tanh(h) for samples jq = 2*pi, 2*pi+1
                th = th_p.tile([128, 2, S], BF16, name="th", tag="th")
                for pi in range(2):
                    for q in range(NQ):
                        ph = ph_p.tile([128, 512], F32, name="ph", tag="ph")
                        nc.tensor.matmul(ph[0:U, :], w1_sb[:],
                                         xt[:, 2 * pi, 512 * q:512 * (q + 1)],
                                         start=True, stop=True)
                        nc.tensor.matmul(ph[U:2 * U, :], w1_sb[:],
                                         xt[:, 2 * pi + 1, 512 * q:512 * (q + 1)],
                                         start=True, stop=True)
                        nc.scalar.activation(
                            th[:, pi, 512 * q:512 * (q + 1)],
                            ph[:], mybir.ActivationFunctionType.Tanh,
                            bias=bb_sb[:])

                # ---- score matmuls: one per (pair, block-row r) ----
                # psc col (r, jq) = 4r + jq
                psc = psc_p.tile([128, 4 * R], F32, name="psc", tag="psc")
                for pi in range(2):
                    thr = th.rearrange("u pi (z r) -> u pi r z", r=R)
                    for r in range(R):
                        nc.tensor.matmul(psc[:, 4 * r + 2 * pi:4 * r + 2 * pi + 2],
                                         thr[:, pi, r, :], w2s_sb[:],
                                         start=True, stop=True)

                # ---- exp, per-sample denominators ----
                e_sb = sm_p.tile([128, 4 * R], F32, name="e_sb", tag="e_sb")
                nc.scalar.activation(e_sb[:], psc[:],
                                     mybir.ActivationFunctionType.Exp)
                cs = sm_p.tile([128, 4], F32, name="cs", tag="cs")
                e_v = e_sb.rearrange("p (r j) -> p r j", j=4)
                for j in range(4):
                    nc.vector.reduce_sum(cs[:, j:j + 1], e_v[:, :, j],
                                         axis=mybir.AxisListType.X)
                pden = pmisc_p.tile([1, 4], F32, name="pden", tag="pden")
                nc.tensor.matmul(pden[:], onc_sb[:], cs[:], start=True, stop=True)
                rec = sm_p.tile([1, 4], F32, name="rec", tag="rec")
                nc.vector.reciprocal(rec[:], pden[:])
                prb = pmisc_p.tile([128, 4], F32, name="prb", tag="prb")
                nc.tensor.matmul(prb[:], onr_sb[:], rec[:], start=True, stop=True)
                rb = sm_p.tile([128, 4], F32, name="rb", tag="rb")
                nc.vector.tensor_copy(rb[:], prb[:])

                # ---- normalize alpha (f32 output + bf16 pooling weights) ----
                alpha_b = sm_p.tile([128, 4 * R], BF16, name="alpha_b",
                                    tag="alpha_b")
                a_v = alpha_b.rearrange("p (r j) -> p r j", j=4)
                for j in range(4):
                    nc.vector.tensor_scalar_mul(
                        alpha_all[:, j, i * R:(i + 1) * R], e_v[:, :, j],
                        rb[:, j:j + 1])
                    nc.vector.tensor_scalar_mul(
                        a_v[:, :, j], e_v[:, :, j], rb[:, j:j + 1])

                # ---- pooling: ctx rows via alpha-as-weights matmuls ----
                pctx = pmisc_p.tile([4, 4 * D], F32, name="pctx", tag="pctx")
                for r in range(R):
                    nc.tensor.matmul(pctx[:], alpha_b[:, 4 * r:4 * r + 4],
                                     xn[:, :, r, :],
                                     start=(r == 0), stop=(r == R - 1))
                nc.vector.tensor_copy(
                    ctx_all[:, i * 4 * D:(i + 1) * 4 * D], pctx[:])

            # ---- alpha out: one DMA per jq over all quads ----
            for j in range(4):
                dst = alp_d.rearrange("(i j) (p r) -> j p i r",
                                      j=4, r=R)[j]
                nc.scalar.dma_start(dst, alpha_all[:, j, :].rearrange(
                    "p (i r) -> p i r", r=R))

            # ---- ctx out: per jq, pick the valid block of each quad ----
            for j in range(4):
                src = ctx_all.rearrange("r (i four d) -> r i four d",
                                        four=4, d=D)[j:j + 1, :, j, :]
                dst = ctx_d.rearrange("(i j) d -> j i d", j=4)[j:j + 1]
                nc.scalar.dma_start(dst, src)

    nc.compile()
    return nc


def make_in_maps(x, W1, W2, b, n_pairs=N_PAIRS, n_cores=N_CORES):
    x = np.asarray(x, dtype=np.float32)
    W1 = np.asarray(W1, dtype=np.float32)
    W2 = np.asarray(W2, dtype=np.float32)
    b = np.asarray(b, dtype=np.float32)

    w1 = W1.astype(ml_dtypes.bfloat16)
    w2s = np.zeros((2 * U, 2), dtype=ml_dtypes.bfloat16)
    w2s[0:U, 0] = W2[:, 0].astype(ml_dtypes.bfloat16)
    w2s[U:2 * U, 1] = W2[:, 0].astype(ml_dtypes.bfloat16)
    bb = np.concatenate([b, b]).reshape(2 * U, 1).astype(np.float32)
    onc = np.ones((128, 1), dtype=np.float32)
    onr = np.ones((1, 128), dtype=np.float32)

    b_local = 2 * n_pairs
    in_maps = []
    for ci in range(n_cores):
        in_maps.append({
            "x": np.ascontiguousarray(x[ci * b_local:(ci + 1) * b_local]),
            "w1": w1, "w2s": w2s, "bb": bb,
            "onc": onc, "onr": onr,
        })
    return in_maps


_CACHE = {}


def _get_graph(n_pairs=N_PAIRS):
    if n_pairs not in _CACHE:
        _CACHE[n_pairs] = build_graph(n_pairs)
    return _CACHE[n_pairs]


def run(x, W1, W2, b, trace=False, n_pairs=N_PAIRS, n_cores=N_CORES, **kw):
    nc = _get_graph(n_pairs)
    in_maps = make_in_maps(x, W1, W2, b, n_pairs, n_cores)
    res = run_bass_kernel_spmd(nc, in_maps, core_ids=list(range(n_cores)),
                               trace=trace, **kw)
    b_local = 2 * n_pairs
    ctx = np.concatenate([np.asarray(r["ctx"]) for r in res.results], axis=0)
    alpha = np.concatenate([np.asarray(r["alpha"]) for r in res.results], axis=0)
    return ctx, alpha.reshape(n_cores * b_local, S, 1), res


def kernel(x, W1, W2, b):
    ctx, alpha, _ = run(x, W1, W2, b)
    return ctx.astype(np.float32), alpha.astype(np.float32)
